# revision 36
# baseline (speedup 1.0000x reference)
"""Trainium2 Bass kernel for nn_Attention (B=4, C=256, L=2048, H=8 heads, D=64).

Sharding: (batch, q-window) across 8 NeuronCores — core j handles batch
j//2 and q columns [(j%2)*1024, (j%2)*1024+1024) for ALL 8 heads:
  - Q is projected for the core's 1024-column q-window (all heads),
  - K and V^T are projected over the full 2048 keys of the core's batch
    (2-way redundant across the two cores sharing a batch — cheap),
  - V^T is produced directly by a matmul with x as the stationary operand
    (no PE transposes), with an appended ones-column so the softmax
    denominator falls out of the PV matmul (M=65),
  - attention runs per head in the S^T (keys-on-partitions) layout,
  - w_out + bias are applied locally (all heads are core-local), so there
    are NO collectives anywhere.
Host reassembles the 8 [256, 1024] column slices into [B, C, L].

The emission is a depth-3 software pipeline over the flat (head,
key-chunk) stream: iteration i emits QK(i+1), exp(i), and PV(i-2), so
the in-order PE queue never blocks on the activation engine and the
activation engine never waits on the PE. Q/K/V^T projection work is
drip-fed in c-half "crumbs" (one per iteration, spaced to fit the
per-iteration PE slack); the first output-projection half accumulates
progressively during the last head so only a short tail remains after
the final softmax normalization. DMA descriptors are few and
need-ordered (each descriptor issues serially on the Sync engine but
its payload fans out across all DMA queues).

All matmul operands fp16 (fp32 PSUM accumulation; matmul outputs are
512 fp32 columns — one PSUM bank — per ISA limit).
"""

import sys

import numpy as np

sys.path.insert(0, "/opt/trn_rl_repo")

import concourse.bass as bass  # noqa: E402
import concourse.bacc as bacc  # noqa: E402
import concourse.tile as tile  # noqa: E402
import concourse.mybir as mybir  # noqa: E402
import concourse.bass_utils as bass_utils  # noqa: E402
from concourse.bass_interp import get_hw_module  # noqa: E402

B, C, L = 4, 256, 2048
H, D = 8, 64
NCORES = 8
QW = 1024                 # q-window per core
NB = 512                  # psum accumulation-group column width (ISA max)
NIT = H * 16              # flat attention iterations (head, key-chunk)
F32 = mybir.dt.float32
F16 = mybir.dt.float16
I16 = mybir.dt.int16
NP16 = np.float16
AF = mybir.ActivationFunctionType
# Schraudolph fp16 exp-by-bit-trick constants for the DVE-offloaded score
# chunks: int16((s + SCH_B/SCH_A) * SCH_A) bitcast to fp16 ~= exp(s). SCH_B
# is calibrated so the approximation is mean-unbiased vs the exact exp
# (the softmax normalization then cancels systematic tilt between chunks).
SCH_A = 1024 * np.log2(np.e)
SCH_B = 15301.625
DVE_KC = (5, 11)  # key chunks per head whose exp runs on the DVE

_CACHE = {}


def _build():
    nc = bacc.Bacc("TRN2", target_bir_lowering=False, debug=False,
                   num_devices=NCORES)

    xb_d = nc.dram_tensor("xb", [128, 2, L], F16, kind="ExternalInput")
    xq_d = nc.dram_tensor("xq", [128, 2, QW], F16, kind="ExternalInput")
    wq_d = nc.dram_tensor("wq", [128, 2, 512], F16, kind="ExternalInput")
    wk_d = nc.dram_tensor("wk", [128, 2, 512], F16, kind="ExternalInput")
    wv_d = nc.dram_tensor("wv", [128, 2, 512], F16, kind="ExternalInput")
    wo_d = nc.dram_tensor("wo", [128, 4, 256], F16, kind="ExternalInput")
    bias_d = nc.dram_tensor("bias2", [128, 2], F32, kind="ExternalInput")
    out = nc.dram_tensor("out", [2, 128, QW], F32, kind="ExternalOutput")

    with tile.TileContext(nc) as tc:
        with (
            tc.tile_pool(name="const", bufs=1) as cpool,
            tc.tile_pool(name="pt", bufs=6) as ptpool,
            tc.tile_pool(name="pti", bufs=2) as pipool,
            tc.tile_pool(name="small", bufs=4) as spool,
            tc.tile_pool(name="psS", bufs=2, space="PSUM") as psS,
            tc.tile_pool(name="psO", bufs=2, space="PSUM") as psO,
        ):
            wq_sb = cpool.tile([128, 2, 512], F16, name="wq_sb")
            wk_sb = cpool.tile([128, 2, 512], F16, name="wk_sb")
            wv_sb = cpool.tile([128, 2, 512], F16, name="wv_sb")
            wo_sb = cpool.tile([128, 4, 256], F16, name="wo_sb")
            bias_sb = cpool.tile([128, 2], F32, name="bias_sb")
            xq_sb = cpool.tile([128, 2, QW], F16, name="xq_sb")
            xb_sb = cpool.tile([128, 2, L], F16, name="xb_sb")
            qd = cpool.tile([128, 4, QW], F16, name="qd")
            kd = cpool.tile([128, 4, L], F16, name="kd")
            vt = cpool.tile([128, 16, 8, 65], F16, name="vt")
            gh = cpool.tile([128, 4, QW], F16, name="gh")

            # DMA descriptors issue serially on the Sync engine (~650ns
            # each) but each descriptor's payload fans out across all DMA
            # queues, so use few, need-ordered descriptors
            nc.sync.dma_start(wq_sb[:], wq_d[:])
            nc.sync.dma_start(xq_sb[:], xq_d[:])
            nc.sync.dma_start(wk_sb[:], wk_d[:])
            nc.sync.dma_start(xb_sb[:, :, 0:256], xb_d[:, :, 0:256])
            nc.sync.dma_start(wv_sb[:], wv_d[:])
            nc.sync.dma_start(xb_sb[:, :, 256:NB], xb_d[:, :, 256:NB])
            nc.sync.dma_start(xb_sb[:, :, NB:L], xb_d[:, :, NB:L])
            nc.sync.dma_start(wo_sb[:], wo_d[:])
            nc.sync.dma_start(bias_sb[:], bias_d[:])
            nc.vector.memset(vt[:, :, :, 64], 1.0)

            def unit_qproj(g):
                """Q projection for head pair g -> qd[:, g, :]."""
                psq = psS.tile([128, 1024], F32, name="psq", tag="psS")
                for c in range(2):
                    for cc in range(2):
                        nc.tensor.matmul(
                            psq[:, c * NB:(c + 1) * NB],
                            wq_sb[:, cc, g * 128:(g + 1) * 128],
                            xq_sb[:, cc, c * NB:(c + 1) * NB],
                            start=(cc == 0), stop=(cc == 1))
                for c in range(2):
                    nc.vector.tensor_copy(
                        qd[:, g, c * NB:(c + 1) * NB],
                        psq[:, c * NB:(c + 1) * NB])

            def unit_kproj(g, half):
                """K projection for head pair g, key half -> kd[:, g, half]."""
                psk = psS.tile([128, 1024], F32, name="psk", tag="psS")
                for c in range(2):
                    for cc in range(2):
                        nc.tensor.matmul(
                            psk[:, c * NB:(c + 1) * NB],
                            wk_sb[:, cc, g * 128:(g + 1) * 128],
                            xb_sb[:, cc, half * 1024 + c * NB:
                                  half * 1024 + (c + 1) * NB],
                            start=(cc == 0), stop=(cc == 1))
                for c in range(2):
                    nc.vector.tensor_copy(
                        kd[:, g, half * 1024 + c * NB:
                           half * 1024 + (c + 1) * NB],
                        psk[:, c * NB:(c + 1) * NB])

            def qproj_c(g, c):
                """One c-half of a Q projection (2 matmuls + cast)."""
                psq = psS.tile([128, 1024], F32, name="psq", tag="psS")
                for cc in range(2):
                    nc.tensor.matmul(
                        psq[:, 0:NB],
                        wq_sb[:, cc, g * 128:(g + 1) * 128],
                        xq_sb[:, cc, c * NB:(c + 1) * NB],
                        start=(cc == 0), stop=(cc == 1))
                nc.vector.tensor_copy(
                    qd[:, g, c * NB:(c + 1) * NB], psq[:, 0:NB])

            def kproj_c(g, half, c):
                """One c-half of a K projection (2 matmuls + cast)."""
                psk = psS.tile([128, 1024], F32, name="psk", tag="psS")
                for cc in range(2):
                    nc.tensor.matmul(
                        psk[:, 0:NB],
                        wk_sb[:, cc, g * 128:(g + 1) * 128],
                        xb_sb[:, cc, half * 1024 + c * NB:
                              half * 1024 + (c + 1) * NB],
                        start=(cc == 0), stop=(cc == 1))
                nc.vector.tensor_copy(
                    kd[:, g, half * 1024 + c * NB:
                       half * 1024 + (c + 1) * NB], psk[:, 0:NB])

            def kproj_cc(g, col0, w):
                """Narrow K projection: kd[:, g, col0:col0+w]."""
                psk = psS.tile([128, 1024], F32, name="psk", tag="psS")
                for cc in range(2):
                    nc.tensor.matmul(
                        psk[:, 0:w],
                        wk_sb[:, cc, g * 128:(g + 1) * 128],
                        xb_sb[:, cc, col0:col0 + w],
                        start=(cc == 0), stop=(cc == 1))
                nc.vector.tensor_copy(
                    kd[:, g, col0:col0 + w], psk[:, 0:w])

            def unit_vt(lc):
                """V^T for key chunk lc (x stationary, w_v moving)."""
                psv = psS.tile([128, 1024], F32, name="psv", tag="psS")
                for cc in range(2):
                    nc.tensor.matmul(
                        psv[:, 0:NB],
                        xb_sb[:, cc, lc * 128:(lc + 1) * 128],
                        wv_sb[:, cc, :],
                        start=(cc == 0), stop=(cc == 1))
                nc.vector.tensor_copy(
                    vt[:, lc, :, 0:64],
                    psv[:, 0:NB].rearrange("p (h d) -> p h d", h=8))

            # drip-fed projection schedule: iteration index -> list of
            # crumb units (each ~2 matmuls + a cast). V^T chunks land
            # just-in-time for their PV; Q/K crumbs for later head pairs
            # are spaced so per-iteration PE slack absorbs them.
            sched = {}

            def at(i, fn, *args):
                sched.setdefault(i, []).append((fn, args))

            # emission deadlines: QK(kc) is emitted at iteration kc-1, so a
            # crumb writing kd for key chunk kc must land at slot <= kc-2;
            # V^T for chunk lc is consumed by PV(lc), emitted at lc+2
            for lc in range(2, 16):
                at(lc - 1, unit_vt, lc)
            at(0, kproj_cc, 0, 256, 256)
            at(1, kproj_c, 0, 0, 1)
            at(4, kproj_c, 0, 1, 0)
            at(8, kproj_c, 0, 1, 1)
            for gi, g in enumerate(range(1, 4)):
                base = (16, 36, 60)[gi]
                step = (3, 4, 5)[gi]
                for ci, crumb in enumerate(
                        [(qproj_c, (g, 0)), (qproj_c, (g, 1)),
                         (kproj_c, (g, 0, 0)), (kproj_c, (g, 0, 1)),
                         (kproj_c, (g, 1, 0)), (kproj_c, (g, 1, 1))]):
                    at(base + ci * step, crumb[0], *crumb[1])

            def emit_qk(i):
                h, kc = divmod(i, 16)
                g, hp = h // 2, h % 2
                p0 = hp * 64
                pss = psS.tile([128, 1024], F32, name="pss", tag="psS")
                for c in range(2):
                    nc.tensor.matmul(
                        pss[:, c * NB:(c + 1) * NB],
                        kd[p0:p0 + 64, g, kc * 128:(kc + 1) * 128],
                        qd[p0:p0 + 64, g, c * NB:(c + 1) * NB],
                        start=True, stop=True)
                return pss

            pso_of = {}

            def emit_norm(h):
                """Normalize head h's PV accumulator into gh, per c-half.

                custom-DVE ops and partition_broadcast mishandle source APs
                with a non-zero partition base: stage the denominator row at
                partition 0 with a plain copy first. The two c-halves are
                pipelined so the gpsimd broadcast overlaps the DVE chain.
                """
                g, hp = h // 2, h % 2
                p0 = hp * 64
                pso = pso_of[h]
                nq = 4 if h == H - 1 else 2      # shorter chain on the tail
                w = 1024 // nq
                bcs = []
                for c in range(nq):
                    cs = slice(c * w, (c + 1) * w)
                    den = spool.tile([1, w], F32, name="den", tag="den")
                    nc.vector.tensor_copy(den[:], pso[64:65, cs])
                    rc = spool.tile([1, w], F32, name="rc", tag="rc")
                    nc.vector.reciprocal_approx_fast(rc[:], den[:])
                    bc = spool.tile([64, w], F32, name="bc", tag="bc")
                    nc.gpsimd.partition_broadcast(bc[:], rc[:])
                    bcs.append(bc)
                for c in range(nq):
                    cs = slice(c * w, (c + 1) * w)
                    nc.vector.tensor_mul(gh[p0:p0 + 64, g, cs],
                                         pso[0:64, cs], bcs[c][:])

            # prologue: minimal critical path to the first exp — Q pair 0
            # and only the first 512 key columns of K gate QK(0)
            unit_qproj(0)
            kproj_cc(0, 0, 256)
            pss_cur = emit_qk(0)
            psy0 = None

            def emit_exp(i, pss_in):
                """Emit exp(i); returns an accessor for the pt slices."""
                h, kc = divmod(i, 16)
                if kc in DVE_KC:
                    pti = pipool.tile([128, 1024], I16, name="pti", tag="pti")
                    nc.vector.tensor_scalar(
                        pti[:], pss_in[:], SCH_B / SCH_A, SCH_A,
                        mybir.AluOpType.add, mybir.AluOpType.mult)
                    return lambda cs: pti[:, cs].bitcast(F16)
                pt = ptpool.tile([128, 1024], F16, name="pt", tag="pt")
                nc.scalar.activation(pt[:], pss_in[:], AF.Exp)
                return lambda cs: pt[:, cs]

            def emit_pv(i, pt_of):
                h, kc = divmod(i, 16)
                for c in range(2):
                    cs = slice(c * NB, (c + 1) * NB)
                    nc.tensor.matmul(
                        pso_of[h][0:65, cs],
                        vt[:, kc, h, :],
                        pt_of(cs),
                        start=(kc == 0), stop=(kc == 15))
                if kc == 15:
                    emit_norm(h)

            # depth-3 software pipeline: each iteration emits QK(i+1),
            # exp(i), and PV(i-2) — the PV's exp finished two iterations
            # ago, so the in-order PE queue never blocks on the activation
            pt_prev = None
            pt_prev2 = None
            for i in range(NIT):
                h, kc = divmod(i, 16)
                if kc == 0:
                    pso_of[h] = psO.tile([128, 1024], F32, name="pso",
                                         tag="pso")
                pss_next = emit_qk(i + 1) if i + 1 < NIT else None
                # crumbs first: their DVE casts gate later QK emissions,
                # so they must precede the V^T casts in the DVE queue
                for fn, args in sched.get(i, ()):
                    fn(*args)
                if i == 0:
                    unit_vt(0)
                    unit_vt(1)
                pt_cur = emit_exp(i, pss_cur)
                if pt_prev2 is not None:
                    emit_pv(i - 2, pt_prev2)
                if h == 7:
                    # progressive out-projection for output channels 0-127:
                    # heads 0-5 are normalized, so accumulate those head
                    # pairs while head 7's attention still runs
                    if kc == 4:
                        psy0 = psO.tile([128, 1024], F32, name="psy0",
                                        tag="pso")
                    if kc in (5, 6, 7):
                        g_acc = kc - 5
                        for c in range(2):
                            nc.tensor.matmul(
                                psy0[:, c * NB:(c + 1) * NB],
                                wo_sb[:, g_acc, 0:128],
                                gh[:, g_acc, c * NB:(c + 1) * NB],
                                start=(g_acc == 0), stop=False)
                pss_cur, pt_prev2, pt_prev = pss_next, pt_prev, pt_cur
            emit_pv(NIT - 2, pt_prev2)
            emit_pv(NIT - 1, pt_prev)

            # tail: psy1's head pairs 0-2 don't depend on head 7 — run them
            # under the final norm chain; the mul(7)-gated closes follow
            psy1 = psS.tile([128, 1024], F32, name="psy1", tag="psS")
            for c in range(2):
                for g in range(3):
                    nc.tensor.matmul(
                        psy1[:, c * NB:(c + 1) * NB],
                        wo_sb[:, g, 128:256],
                        gh[:, g, c * NB:(c + 1) * NB],
                        start=(g == 0), stop=False)
            y0 = spool.tile([128, 1024], F32, name="y0", tag="y")
            y1 = spool.tile([128, 1024], F32, name="y1", tag="y")
            for c in range(2):
                cs = slice(c * NB, (c + 1) * NB)
                nc.tensor.matmul(
                    psy0[:, cs], wo_sb[:, 3, 0:128], gh[:, 3, cs],
                    start=False, stop=True)
                nc.tensor.matmul(
                    psy1[:, cs], wo_sb[:, 3, 128:256], gh[:, 3, cs],
                    start=False, stop=True)
                nc.scalar.activation(y0[:, cs], psy0[:, cs], AF.Identity,
                                     bias=bias_sb[:, 0:1])
                nc.sync.dma_start(out[0][:, cs], y0[:, cs])
                nc.vector.tensor_scalar_add(y1[:, cs], psy1[:, cs],
                                            bias_sb[:, 1:2])
                nc.sync.dma_start(out[1][:, cs], y1[:, cs])

    nc.compile()
    nc.m = get_hw_module(nc.m)
    return nc


def _prep_in_maps(x, w_qkv, w_out, b_out):
    scale = float(D) ** -0.5
    x = np.asarray(x, np.float32)
    w_qkv = np.asarray(w_qkv, np.float32)
    w_out = np.asarray(w_out, np.float32)
    b_out = np.asarray(b_out, np.float32)

    x16 = x.astype(NP16)                            # [4, 256, 2048]

    def pack_w(w):
        # w [512 out, 256 c] -> [128 (c%128), 2 (c//128), 512 out]
        return np.ascontiguousarray(
            w.T.reshape(2, 128, 512).transpose(1, 0, 2)).astype(NP16)

    wq_p = pack_w(w_qkv[0:512] * scale)
    wk_p = pack_w(w_qkv[512:1024])
    wv_p = pack_w(w_qkv[1024:1536])
    wo_p = np.ascontiguousarray(
        w_out.T.reshape(4, 128, 256).transpose(1, 0, 2)).astype(NP16)
    bias2 = np.ascontiguousarray(b_out.reshape(2, 128).T)

    in_maps = []
    for j in range(NCORES):
        b, q0 = j // 2, (j % 2) * QW
        xb = np.ascontiguousarray(
            x16[b].reshape(2, 128, L).transpose(1, 0, 2))   # [128, 2, L]
        xq = np.ascontiguousarray(xb[:, :, q0:q0 + QW])
        in_maps.append({"xb": xb, "xq": xq, "wq": wq_p, "wk": wk_p,
                        "wv": wv_p, "wo": wo_p, "bias2": bias2})
    return in_maps


def _run(inputs, trace=False):
    if "nc" not in _CACHE:
        _CACHE["nc"] = _build()
    nc = _CACHE["nc"]
    in_maps = _prep_in_maps(**inputs)
    res = bass_utils.run_bass_kernel_spmd(
        nc, in_maps, core_ids=list(range(NCORES)), trace=trace)
    y = np.empty((B, C, L), np.float32)
    for j in range(NCORES):
        b, q0 = j // 2, (j % 2) * QW
        o = res.results[j]["out"]                   # [2, 128, QW]
        y[b, 0:128, q0:q0 + QW] = o[0]
        y[b, 128:256, q0:q0 + QW] = o[1]
    return y, res


def kernel(x, w_qkv, w_out, b_out):
    y, _ = _run(dict(x=x, w_qkv=w_qkv, w_out=w_out, b_out=b_out), trace=False)
    return y


# revision 37
# speedup vs baseline: 1.0244x; 1.0244x over previous
"""Trainium2 Bass kernel for nn_Attention (B=4, C=256, L=2048, H=8 heads, D=64).

Sharding: (batch, q-window) across 8 NeuronCores — core j handles batch
j//2 and q columns [(j%2)*1024, (j%2)*1024+1024) for ALL 8 heads:
  - Q is projected for the core's 1024-column q-window (all heads),
  - K and V^T are projected over the full 2048 keys of the core's batch
    (2-way redundant across the two cores sharing a batch — cheap),
  - V^T is produced directly by a matmul with x as the stationary operand
    (no PE transposes), with an appended ones-column so the softmax
    denominator falls out of the PV matmul (M=65),
  - attention runs per head in the S^T (keys-on-partitions) layout,
  - w_out + bias are applied locally (all heads are core-local), so there
    are NO collectives anywhere.
Host reassembles the 8 [256, 1024] column slices into [B, C, L].

The emission is a depth-3 software pipeline over the flat (head,
key-chunk) stream: iteration i emits QK(i+1), exp(i), and PV(i-2), so
the in-order PE queue never blocks on the activation engine and the
activation engine never waits on the PE. Q/K/V^T projection work is
drip-fed in c-half "crumbs" (one per iteration, spaced to fit the
per-iteration PE slack); the first output-projection half accumulates
progressively during the last head so only a short tail remains after
the final softmax normalization. DMA descriptors are few and
need-ordered (each descriptor issues serially on the Sync engine but
its payload fans out across all DMA queues).

All matmul operands fp16 (fp32 PSUM accumulation; matmul outputs are
512 fp32 columns — one PSUM bank — per ISA limit).
"""

import sys

import numpy as np

sys.path.insert(0, "/opt/trn_rl_repo")

import concourse.bass as bass  # noqa: E402
import concourse.bacc as bacc  # noqa: E402
import concourse.tile as tile  # noqa: E402
import concourse.mybir as mybir  # noqa: E402
import concourse.bass_utils as bass_utils  # noqa: E402
from concourse.bass_interp import get_hw_module  # noqa: E402

B, C, L = 4, 256, 2048
H, D = 8, 64
NCORES = 8
QW = 1024                 # q-window per core
NB = 512                  # psum accumulation-group column width (ISA max)
NIT = H * 16              # flat attention iterations (head, key-chunk)
F32 = mybir.dt.float32
F16 = mybir.dt.float16
I16 = mybir.dt.int16
NP16 = np.float16
AF = mybir.ActivationFunctionType
# Schraudolph fp16 exp-by-bit-trick constants for the DVE-offloaded score
# chunks: int16((s + SCH_B/SCH_A) * SCH_A) bitcast to fp16 ~= exp(s). SCH_B
# is calibrated so the approximation is mean-unbiased vs the exact exp
# (the softmax normalization then cancels systematic tilt between chunks).
SCH_A = 1024 * np.log2(np.e)
SCH_B = 15301.625
DVE_KC = ()       # key chunks per head whose exp runs on the DVE

_CACHE = {}


def _build():
    nc = bacc.Bacc("TRN2", target_bir_lowering=False, debug=False,
                   num_devices=NCORES)

    xb_d = nc.dram_tensor("xb", [128, 2, L], F16, kind="ExternalInput")
    xq_d = nc.dram_tensor("xq", [128, 2, QW], F16, kind="ExternalInput")
    wq_d = nc.dram_tensor("wq", [128, 2, 512], F16, kind="ExternalInput")
    wk_d = nc.dram_tensor("wk", [128, 2, 512], F16, kind="ExternalInput")
    wv_d = nc.dram_tensor("wv", [128, 2, 512], F16, kind="ExternalInput")
    wo_d = nc.dram_tensor("wo", [128, 4, 256], F16, kind="ExternalInput")
    bias_d = nc.dram_tensor("bias2", [128, 2], F32, kind="ExternalInput")
    out = nc.dram_tensor("out", [2, 128, QW], F32, kind="ExternalOutput")

    with tile.TileContext(nc) as tc:
        with (
            tc.tile_pool(name="const", bufs=1) as cpool,
            tc.tile_pool(name="pt", bufs=6) as ptpool,
            tc.tile_pool(name="pti", bufs=2) as pipool,
            tc.tile_pool(name="small", bufs=4) as spool,
            tc.tile_pool(name="psS", bufs=2, space="PSUM") as psS,
            tc.tile_pool(name="psO", bufs=2, space="PSUM") as psO,
        ):
            wq_sb = cpool.tile([128, 2, 512], F16, name="wq_sb")
            wk_sb = cpool.tile([128, 2, 512], F16, name="wk_sb")
            wv_sb = cpool.tile([128, 2, 512], F16, name="wv_sb")
            wo_sb = cpool.tile([128, 4, 256], F16, name="wo_sb")
            bias_sb = cpool.tile([128, 2], F32, name="bias_sb")
            xq_sb = cpool.tile([128, 2, QW], F16, name="xq_sb")
            xb_sb = cpool.tile([128, 2, L], F16, name="xb_sb")
            qd = cpool.tile([128, 4, QW], F16, name="qd")
            kd = cpool.tile([128, 4, L], F16, name="kd")
            vt = cpool.tile([128, 16, 8, 65], F16, name="vt")
            gh = cpool.tile([128, 4, QW], F16, name="gh")

            # DMA descriptors issue serially on the Sync engine (~650ns
            # each) but each descriptor's payload fans out across all DMA
            # queues, so use few, need-ordered descriptors
            nc.sync.dma_start(wq_sb[:], wq_d[:])
            nc.sync.dma_start(xq_sb[:], xq_d[:])
            nc.sync.dma_start(wk_sb[:], wk_d[:])
            nc.sync.dma_start(xb_sb[:, :, 0:256], xb_d[:, :, 0:256])
            nc.sync.dma_start(wv_sb[:], wv_d[:])
            nc.sync.dma_start(xb_sb[:, :, 256:NB], xb_d[:, :, 256:NB])
            nc.sync.dma_start(xb_sb[:, :, NB:L], xb_d[:, :, NB:L])
            nc.sync.dma_start(wo_sb[:], wo_d[:])
            nc.sync.dma_start(bias_sb[:], bias_d[:])
            nc.vector.memset(vt[:, :, :, 64], 1.0)

            def unit_qproj(g):
                """Q projection for head pair g -> qd[:, g, :]."""
                psq = psS.tile([128, 1024], F32, name="psq", tag="psS")
                for c in range(2):
                    for cc in range(2):
                        nc.tensor.matmul(
                            psq[:, c * NB:(c + 1) * NB],
                            wq_sb[:, cc, g * 128:(g + 1) * 128],
                            xq_sb[:, cc, c * NB:(c + 1) * NB],
                            start=(cc == 0), stop=(cc == 1))
                for c in range(2):
                    nc.vector.tensor_copy(
                        qd[:, g, c * NB:(c + 1) * NB],
                        psq[:, c * NB:(c + 1) * NB])

            def unit_kproj(g, half):
                """K projection for head pair g, key half -> kd[:, g, half]."""
                psk = psS.tile([128, 1024], F32, name="psk", tag="psS")
                for c in range(2):
                    for cc in range(2):
                        nc.tensor.matmul(
                            psk[:, c * NB:(c + 1) * NB],
                            wk_sb[:, cc, g * 128:(g + 1) * 128],
                            xb_sb[:, cc, half * 1024 + c * NB:
                                  half * 1024 + (c + 1) * NB],
                            start=(cc == 0), stop=(cc == 1))
                for c in range(2):
                    nc.vector.tensor_copy(
                        kd[:, g, half * 1024 + c * NB:
                           half * 1024 + (c + 1) * NB],
                        psk[:, c * NB:(c + 1) * NB])

            def qproj_c(g, c):
                """One c-half of a Q projection (2 matmuls + cast)."""
                psq = psS.tile([128, 1024], F32, name="psq", tag="psS")
                for cc in range(2):
                    nc.tensor.matmul(
                        psq[:, 0:NB],
                        wq_sb[:, cc, g * 128:(g + 1) * 128],
                        xq_sb[:, cc, c * NB:(c + 1) * NB],
                        start=(cc == 0), stop=(cc == 1))
                nc.vector.tensor_copy(
                    qd[:, g, c * NB:(c + 1) * NB], psq[:, 0:NB])

            def kproj_c(g, half, c):
                """One c-half of a K projection (2 matmuls + cast)."""
                psk = psS.tile([128, 1024], F32, name="psk", tag="psS")
                for cc in range(2):
                    nc.tensor.matmul(
                        psk[:, 0:NB],
                        wk_sb[:, cc, g * 128:(g + 1) * 128],
                        xb_sb[:, cc, half * 1024 + c * NB:
                              half * 1024 + (c + 1) * NB],
                        start=(cc == 0), stop=(cc == 1))
                nc.vector.tensor_copy(
                    kd[:, g, half * 1024 + c * NB:
                       half * 1024 + (c + 1) * NB], psk[:, 0:NB])

            def kproj_cc(g, col0, w):
                """Narrow K projection: kd[:, g, col0:col0+w]."""
                psk = psS.tile([128, 1024], F32, name="psk", tag="psS")
                for cc in range(2):
                    nc.tensor.matmul(
                        psk[:, 0:w],
                        wk_sb[:, cc, g * 128:(g + 1) * 128],
                        xb_sb[:, cc, col0:col0 + w],
                        start=(cc == 0), stop=(cc == 1))
                nc.vector.tensor_copy(
                    kd[:, g, col0:col0 + w], psk[:, 0:w])

            def unit_vt(lc):
                """V^T for key chunk lc (x stationary, w_v moving)."""
                psv = psS.tile([128, 1024], F32, name="psv", tag="psS")
                for cc in range(2):
                    nc.tensor.matmul(
                        psv[:, 0:NB],
                        xb_sb[:, cc, lc * 128:(lc + 1) * 128],
                        wv_sb[:, cc, :],
                        start=(cc == 0), stop=(cc == 1))
                nc.vector.tensor_copy(
                    vt[:, lc, :, 0:64],
                    psv[:, 0:NB].rearrange("p (h d) -> p h d", h=8))

            # drip-fed projection schedule: iteration index -> list of
            # crumb units (each ~2 matmuls + a cast). V^T chunks land
            # just-in-time for their PV; Q/K crumbs for later head pairs
            # are spaced so per-iteration PE slack absorbs them.
            sched = {}

            def at(i, fn, *args):
                sched.setdefault(i, []).append((fn, args))

            # emission deadlines: QK(kc) is emitted at iteration kc-1, so a
            # crumb writing kd for key chunk kc must land at slot <= kc-2;
            # V^T for chunk lc is consumed by PV(lc), emitted at lc+2
            for lc in range(2, 16):
                at(lc - 1, unit_vt, lc)
            at(0, kproj_cc, 0, 256, 256)
            at(1, kproj_c, 0, 0, 1)
            at(4, kproj_c, 0, 1, 0)
            at(8, kproj_c, 0, 1, 1)
            for gi, g in enumerate(range(1, 4)):
                base = (16, 36, 60)[gi]
                step = (3, 4, 5)[gi]
                for ci, crumb in enumerate(
                        [(qproj_c, (g, 0)), (qproj_c, (g, 1)),
                         (kproj_c, (g, 0, 0)), (kproj_c, (g, 0, 1)),
                         (kproj_c, (g, 1, 0)), (kproj_c, (g, 1, 1))]):
                    at(base + ci * step, crumb[0], *crumb[1])

            def emit_qk(i):
                h, kc = divmod(i, 16)
                g, hp = h // 2, h % 2
                p0 = hp * 64
                pss = psS.tile([128, 1024], F32, name="pss", tag="psS")
                for c in range(2):
                    nc.tensor.matmul(
                        pss[:, c * NB:(c + 1) * NB],
                        kd[p0:p0 + 64, g, kc * 128:(kc + 1) * 128],
                        qd[p0:p0 + 64, g, c * NB:(c + 1) * NB],
                        start=True, stop=True)
                return pss

            pso_of = {}

            def emit_norm(h):
                """Normalize head h's PV accumulator into gh, per c-half.

                custom-DVE ops and partition_broadcast mishandle source APs
                with a non-zero partition base: stage the denominator row at
                partition 0 with a plain copy first. The two c-halves are
                pipelined so the gpsimd broadcast overlaps the DVE chain.
                """
                g, hp = h // 2, h % 2
                p0 = hp * 64
                pso = pso_of[h]
                nq = 4 if h == H - 1 else 2      # shorter chain on the tail
                w = 1024 // nq
                bcs = []
                for c in range(nq):
                    cs = slice(c * w, (c + 1) * w)
                    den = spool.tile([1, w], F32, name="den", tag="den")
                    nc.vector.tensor_copy(den[:], pso[64:65, cs])
                    rc = spool.tile([1, w], F32, name="rc", tag="rc")
                    nc.vector.reciprocal_approx_fast(rc[:], den[:])
                    bc = spool.tile([64, w], F32, name="bc", tag="bc")
                    nc.gpsimd.partition_broadcast(bc[:], rc[:])
                    bcs.append(bc)
                for c in range(nq):
                    cs = slice(c * w, (c + 1) * w)
                    nc.vector.tensor_mul(gh[p0:p0 + 64, g, cs],
                                         pso[0:64, cs], bcs[c][:])

            # prologue: minimal critical path to the first exp — Q pair 0
            # and only the first 512 key columns of K gate QK(0)
            unit_qproj(0)
            kproj_cc(0, 0, 256)
            pss_cur = emit_qk(0)
            psy0 = None

            def emit_exp(i, pss_in):
                """Emit exp(i); returns an accessor for the pt slices."""
                h, kc = divmod(i, 16)
                if kc in DVE_KC:
                    pti = pipool.tile([128, 1024], I16, name="pti", tag="pti")
                    nc.vector.tensor_scalar(
                        pti[:], pss_in[:], SCH_B / SCH_A, SCH_A,
                        mybir.AluOpType.add, mybir.AluOpType.mult)
                    return lambda cs: pti[:, cs].bitcast(F16)
                pt = ptpool.tile([128, 1024], F16, name="pt", tag="pt")
                nc.scalar.activation(pt[:], pss_in[:], AF.Exp)
                return lambda cs: pt[:, cs]

            def emit_pv(i, pt_of):
                h, kc = divmod(i, 16)
                for c in range(2):
                    cs = slice(c * NB, (c + 1) * NB)
                    nc.tensor.matmul(
                        pso_of[h][0:65, cs],
                        vt[:, kc, h, :],
                        pt_of(cs),
                        start=(kc == 0), stop=(kc == 15))
                if kc == 15:
                    emit_norm(h)

            # depth-3 software pipeline: each iteration emits QK(i+1),
            # exp(i), and PV(i-2) — the PV's exp finished two iterations
            # ago, so the in-order PE queue never blocks on the activation
            pt_prev = None
            pt_prev2 = None
            for i in range(NIT):
                h, kc = divmod(i, 16)
                if kc == 0:
                    pso_of[h] = psO.tile([128, 1024], F32, name="pso",
                                         tag="pso")
                pss_next = emit_qk(i + 1) if i + 1 < NIT else None
                # crumbs first: their DVE casts gate later QK emissions,
                # so they must precede the V^T casts in the DVE queue
                for fn, args in sched.get(i, ()):
                    fn(*args)
                if i == 0:
                    unit_vt(0)
                    unit_vt(1)
                pt_cur = emit_exp(i, pss_cur)
                if pt_prev2 is not None:
                    emit_pv(i - 2, pt_prev2)
                if h == 7:
                    # progressive out-projection for output channels 0-127:
                    # heads 0-5 are normalized, so accumulate those head
                    # pairs while head 7's attention still runs
                    if kc == 4:
                        psy0 = psO.tile([128, 1024], F32, name="psy0",
                                        tag="pso")
                    if kc in (5, 6, 7):
                        g_acc = kc - 5
                        for c in range(2):
                            nc.tensor.matmul(
                                psy0[:, c * NB:(c + 1) * NB],
                                wo_sb[:, g_acc, 0:128],
                                gh[:, g_acc, c * NB:(c + 1) * NB],
                                start=(g_acc == 0), stop=False)
                pss_cur, pt_prev2, pt_prev = pss_next, pt_prev, pt_cur
            emit_pv(NIT - 2, pt_prev2)
            emit_pv(NIT - 1, pt_prev)

            # tail: psy1's head pairs 0-2 don't depend on head 7 — run them
            # under the final norm chain; the mul(7)-gated closes follow
            psy1 = psS.tile([128, 1024], F32, name="psy1", tag="psS")
            for c in range(2):
                for g in range(3):
                    nc.tensor.matmul(
                        psy1[:, c * NB:(c + 1) * NB],
                        wo_sb[:, g, 128:256],
                        gh[:, g, c * NB:(c + 1) * NB],
                        start=(g == 0), stop=False)
            y0 = spool.tile([128, 1024], F32, name="y0", tag="y")
            y1 = spool.tile([128, 1024], F32, name="y1", tag="y")
            for c in range(2):
                cs = slice(c * NB, (c + 1) * NB)
                nc.tensor.matmul(
                    psy0[:, cs], wo_sb[:, 3, 0:128], gh[:, 3, cs],
                    start=False, stop=True)
                nc.tensor.matmul(
                    psy1[:, cs], wo_sb[:, 3, 128:256], gh[:, 3, cs],
                    start=False, stop=True)
                nc.scalar.activation(y0[:, cs], psy0[:, cs], AF.Identity,
                                     bias=bias_sb[:, 0:1])
                nc.sync.dma_start(out[0][:, cs], y0[:, cs])
                nc.vector.tensor_scalar_add(y1[:, cs], psy1[:, cs],
                                            bias_sb[:, 1:2])
                nc.sync.dma_start(out[1][:, cs], y1[:, cs])

    nc.compile()
    nc.m = get_hw_module(nc.m)
    return nc


def _prep_in_maps(x, w_qkv, w_out, b_out):
    scale = float(D) ** -0.5
    x = np.asarray(x, np.float32)
    w_qkv = np.asarray(w_qkv, np.float32)
    w_out = np.asarray(w_out, np.float32)
    b_out = np.asarray(b_out, np.float32)

    x16 = x.astype(NP16)                            # [4, 256, 2048]

    def pack_w(w):
        # w [512 out, 256 c] -> [128 (c%128), 2 (c//128), 512 out]
        return np.ascontiguousarray(
            w.T.reshape(2, 128, 512).transpose(1, 0, 2)).astype(NP16)

    wq_p = pack_w(w_qkv[0:512] * scale)
    wk_p = pack_w(w_qkv[512:1024])
    wv_p = pack_w(w_qkv[1024:1536])
    wo_p = np.ascontiguousarray(
        w_out.T.reshape(4, 128, 256).transpose(1, 0, 2)).astype(NP16)
    bias2 = np.ascontiguousarray(b_out.reshape(2, 128).T)

    in_maps = []
    for j in range(NCORES):
        b, q0 = j // 2, (j % 2) * QW
        xb = np.ascontiguousarray(
            x16[b].reshape(2, 128, L).transpose(1, 0, 2))   # [128, 2, L]
        xq = np.ascontiguousarray(xb[:, :, q0:q0 + QW])
        in_maps.append({"xb": xb, "xq": xq, "wq": wq_p, "wk": wk_p,
                        "wv": wv_p, "wo": wo_p, "bias2": bias2})
    return in_maps


def _run(inputs, trace=False):
    if "nc" not in _CACHE:
        _CACHE["nc"] = _build()
    nc = _CACHE["nc"]
    in_maps = _prep_in_maps(**inputs)
    res = bass_utils.run_bass_kernel_spmd(
        nc, in_maps, core_ids=list(range(NCORES)), trace=trace)
    y = np.empty((B, C, L), np.float32)
    for j in range(NCORES):
        b, q0 = j // 2, (j % 2) * QW
        o = res.results[j]["out"]                   # [2, 128, QW]
        y[b, 0:128, q0:q0 + QW] = o[0]
        y[b, 128:256, q0:q0 + QW] = o[1]
    return y, res


def kernel(x, w_qkv, w_out, b_out):
    y, _ = _run(dict(x=x, w_qkv=w_qkv, w_out=w_out, b_out=b_out), trace=False)
    return y
